# revision 8
# baseline (speedup 1.0000x reference)
"""Causal multi-head attention block (B=4, T=2048, C=1024, H=16, D=64) on 8 trn2 cores.

Sharding: core c handles batch b = c // 2 and heads g*8..g*8+8 where g = c % 2.
Each core computes the QKV projection restricted to its 8 heads, causal
attention, and a partial out-projection (contracting only its 512 head-dims).
The host sums the two partials per batch and adds the output bias.

On-device layout: attention is computed transposed -- S^T[k, q] = K @ Q^T --
so that the softmax sum over keys (the partition dim of S^T) folds into the
P@V matmul via an appended ones-column on V, and the attention output lands
as y^T [head_dim, T], which is exactly the lhsT layout the out-projection
matmul needs. Normalization uses a rank-1 outer-product broadcast of the
reciprocal denominators. Matmuls run in float32r (full PE rate at N>=256).
"""

import sys
from contextlib import ExitStack

import numpy as np

sys.path.insert(0, "/opt/trn_rl_repo")

import concourse.bass as bass  # noqa: F401
import concourse.mybir as mybir
import concourse.tile as tile
from concourse import bacc
from concourse.bass_interp import get_hw_module
from concourse.bass_utils import run_bass_kernel_spmd
from concourse.masks import make_identity

F32 = mybir.dt.float32
F32R = mybir.dt.float32r
AF = mybir.ActivationFunctionType

B, T, C, H, D = 4, 2048, 1024, 16, 64
HPC = 8            # heads per core
CPC = HPC * D      # 512 head-dim columns per core
N_CORES = 8
TB = T // 128      # 16 T row-blocks
KT = C // 128      # 8 contraction tiles over C
NQ = T // 512      # 4 q-chunks


def _emit(ctx, tc, xs, wqkv, bqkv, wout, ones_d, tri_d, out):
    nc = tc.nc

    const = ctx.enter_context(tc.tile_pool(name="const", bufs=1))
    identity = const.tile([128, 128], F32)
    make_identity(nc, identity)
    # tri[k, q] = 1.0 where k <= q else 0.0 (valid region of the causal mask in
    # transposed-score coordinates: keys on partitions, queries on free dim).
    # Loaded from the host: f32r-consumed constants must come from DMA.
    tri = const.tile([128, 128], F32R)
    nc.sync.dma_start(out=tri, in_=tri_d[:, :].bitcast(F32R))
    ones_s = const.tile([1, 128], F32R)
    nc.sync.dma_start(out=ones_s, in_=ones_d[0:1, :].bitcast(F32R))
    bias_qk = const.tile([128, 8], F32)
    nc.sync.dma_start(out=bias_qk, in_=bqkv[0 : 2 * CPC].rearrange("(m p) -> p m", p=128))
    bias_v = const.tile([1, CPC], F32R)
    nc.sync.dma_start(out=bias_v, in_=bqkv[2 * CPC :].rearrange("(a m) -> a m", a=1).bitcast(F32R))

    # Big reusable region: holds x^T during phases A/B, then y^T + W_out after.
    big_pool = ctx.enter_context(tc.tile_pool(name="big", bufs=1))
    # qk^T [1024 rows, T] as [128, 8, T]; rows 0..511 = q^T, 512..1023 = k^T
    qkT = ctx.enter_context(tc.tile_pool(name="qkT", bufs=1)).tile([128, 8, T], F32R)
    # v natural [T, 8 heads, 64+1] as [128, TB, 8, 65]; col 64 = ones
    v_all = ctx.enter_context(tc.tile_pool(name="vall", bufs=1)).tile(
        [128, TB, HPC, D + 1], F32R
    )

    xt = big_pool.tile([128, KT, T], F32R, tag="big", name="xt")

    # ---------- Phase A: load x and build x^T ----------
    with (
        tc.tile_pool(name="xnat", bufs=3) as xnat_pool,
        tc.tile_pool(name="ps_t", bufs=4, space="PSUM") as ps_t_pool,
    ):
        for tb in range(TB):
            xn = xnat_pool.tile([128, C], F32)
            nc.sync.dma_start(out=xn, in_=xs[tb * 128 : (tb + 1) * 128, :])
            for cb in range(KT):
                pst = ps_t_pool.tile([128, 128], F32)
                nc.tensor.transpose(pst, xn[:, cb * 128 : (cb + 1) * 128], identity)
                dest = xt[:, cb, tb * 128 : (tb + 1) * 128]
                if cb % 2 == 0:
                    nc.vector.tensor_copy(dest, pst)
                else:
                    nc.scalar.activation(dest, pst, AF.Copy)

    # ---------- Phase B: QKV projection ----------
    with (
        tc.tile_pool(name="wst", bufs=16) as w_pool,
        tc.tile_pool(name="wv", bufs=1) as wv_pool,
        tc.tile_pool(name="ps_mm", bufs=4, space="PSUM") as ps_mm_pool,
    ):
        # q^T / k^T: out rows = qk column index (per-partition), streamed over T
        for mb in range(8):
            w_tiles = []
            for kt in range(KT):
                w_t = w_pool.tile([128, 128], F32R, tag="w")
                nc.sync.dma_start(
                    out=w_t,
                    in_=wqkv[kt * 128 : (kt + 1) * 128, mb * 128 : (mb + 1) * 128].bitcast(F32R),
                )
                w_tiles.append(w_t)
            for nch in range(NQ):
                ps = ps_mm_pool.tile([128, 512], F32)
                for kt in range(KT):
                    nc.tensor.matmul(
                        ps,
                        w_tiles[kt],
                        xt[:, kt, nch * 512 : (nch + 1) * 512],
                        start=(kt == 0),
                        stop=(kt == KT - 1),
                    )
                nc.scalar.activation(
                    qkT[:, mb, nch * 512 : (nch + 1) * 512],
                    ps,
                    AF.Identity,
                    bias=bias_qk[:, mb : mb + 1],
                )

        # v natural: rows = keys, cols = 8 heads x 64; bias added via K=1 matmul
        wv = wv_pool.tile([128, KT, CPC], F32R)
        for kt in range(KT):
            nc.sync.dma_start(
                out=wv[:, kt, :],
                in_=wqkv[kt * 128 : (kt + 1) * 128, 2 * CPC : 3 * CPC].bitcast(F32R),
            )
        nc.sync.dma_start(
            out=v_all[:, :, :, D : D + 1],
            in_=ones_d[:, 0 : TB * HPC]
            .rearrange("p (a b c) -> p a b c", a=TB, c=1)
            .bitcast(F32R),
        )
        for tb in range(TB):
            ps = ps_mm_pool.tile([128, 512], F32)
            for kt in range(KT):
                nc.tensor.matmul(
                    ps,
                    xt[:, kt, tb * 128 : (tb + 1) * 128],
                    wv[:, kt, :],
                    start=(kt == 0),
                    stop=False,
                )
            nc.tensor.matmul(
                ps, ones_s, bias_v, start=False, stop=True
            )
            nc.vector.tensor_copy(
                v_all[:, tb, :, 0:D], ps[:].rearrange("p (h d) -> p h d", h=HPC)
            )

    # x^T region is dead now; reuse it for y^T (rows = 512 head dims) + W_out.
    # combo[:, i, 0:T] = y^T tile i; combo[:, i, T:T+C] = W_out rows i*128..
    combo = big_pool.tile([128, 4, T + C], F32R, tag="big", name="combo")
    for kt in range(4):
        nc.sync.dma_start(
            out=combo[:, kt, T : T + C], in_=wout[kt * 128 : (kt + 1) * 128, :].bitcast(F32R)
        )

    # ---------- Phase C: attention + out-projection, interleaved per q-chunk ----------
    with (
        tc.tile_pool(name="e", bufs=4) as e_pool,
        tc.tile_pool(name="bc", bufs=2) as bc_pool,
        tc.tile_pool(name="rcp", bufs=2) as rcp_pool,
        tc.tile_pool(name="ost", bufs=4) as ost_pool,
        tc.tile_pool(name="ps_s", bufs=2, space="PSUM") as ps_s_pool,
        tc.tile_pool(name="ps_y", bufs=2, space="PSUM") as ps_y_pool,
        tc.tile_pool(name="ps_b", bufs=2, space="PSUM") as ps_b_pool,
        tc.tile_pool(name="ps_o", bufs=2, space="PSUM") as ps_o_pool,
    ):
        for qc in range(NQ):
            q0 = qc * 512
            for h in range(HPC):
                po = (h % 2) * 64  # partition offset of this head's 64 rows
                qi = h // 2        # q^T tile index; k^T tile index is 4 + qi
                qT_row = lambda c0, c1: qkT[po : po + 64, qi, c0:c1]  # noqa: E731
                kT_row = lambda c0, c1: qkT[po : po + 64, 4 + qi, c0:c1]  # noqa: E731

                py = ps_y_pool.tile([65, 512], F32)
                n_kb = qc * 4 + 4
                for kb in range(n_kb):
                    dd = kb - qc * 4  # >= 0 on diagonal blocks
                    e_sb = e_pool.tile([128, 512], F32R, tag="e")
                    ps_s = ps_s_pool.tile([128, 512], F32)
                    if dd < 0:
                        # full block: all 512 queries attend to these 128 keys
                        nc.tensor.matmul(
                            ps_s,
                            kT_row(kb * 128, kb * 128 + 128),
                            qT_row(q0, q0 + 512),
                            start=True,
                            stop=True,
                        )
                        nc.scalar.activation(e_sb, ps_s, AF.Exp, scale=0.125)
                        nc.tensor.matmul(
                            py,
                            v_all[:, kb, h, :],
                            e_sb,
                            start=(kb == 0),
                            stop=(kb == n_kb - 1),
                        )
                    else:
                        # diagonal block: queries below dd*128 are fully masked
                        c0 = dd * 128
                        nc.tensor.matmul(
                            ps_s[:, c0:],
                            kT_row(kb * 128, kb * 128 + 128),
                            qT_row(q0 + c0, q0 + 512),
                            start=True,
                            stop=True,
                        )
                        nc.scalar.activation(
                            e_sb[:, c0:], ps_s[:, c0:], AF.Exp, scale=0.125
                        )
                        nc.vector.tensor_mul(
                            e_sb[:, c0 : c0 + 128], e_sb[:, c0 : c0 + 128], tri
                        )
                        nc.tensor.matmul(
                            py[:, c0:],
                            v_all[:, kb, h, :],
                            e_sb[:, c0:],
                            start=(kb == 0),
                            stop=(kb == n_kb - 1),
                        )

                # normalize: y^T[d, q] = py[d, q] * (1 / py[64, q])
                rcp = rcp_pool.tile([1, 512], F32R)
                with nc.allow_low_precision(reason="f32r denominators for matmul"):
                    nc.vector.reciprocal(rcp, py[64:65, :])
                pb = ps_b_pool.tile([64, 512], F32)
                nc.tensor.matmul(pb, ones_s[0:1, 0:64], rcp, start=True, stop=True)
                bc = bc_pool.tile([64, 512], F32)
                nc.vector.tensor_copy(bc, pb)
                nc.vector.tensor_mul(
                    combo[po : po + 64, qi, q0 : q0 + 512], py[0:64, :], bc
                )

            # out-projection for the 4 T-blocks covered by this q-chunk
            for tb in range(qc * 4, qc * 4 + 4):
                for nn in range(2):
                    ps_o = ps_o_pool.tile([128, 512], F32)
                    for kt in range(4):
                        nc.tensor.matmul(
                            ps_o,
                            combo[:, kt, tb * 128 : (tb + 1) * 128],
                            combo[:, kt, T + nn * 512 : T + (nn + 1) * 512],
                            start=(kt == 0),
                            stop=(kt == 3),
                        )
                    o_sb = ost_pool.tile([128, 512], F32, tag="o")
                    if nn == 0:
                        nc.scalar.activation(o_sb, ps_o, AF.Copy)
                    else:
                        nc.vector.tensor_copy(o_sb, ps_o)
                    nc.sync.dma_start(
                        out=out[tb * 128 : (tb + 1) * 128, nn * 512 : (nn + 1) * 512],
                        in_=o_sb,
                    )


_COMPILED = {}


def _get_compiled():
    if "nc" not in _COMPILED:
        nc = bacc.Bacc(
            "TRN2",
            target_bir_lowering=False,
            debug=False,
            num_devices=N_CORES,
        )
        xs = nc.dram_tensor("xs", [T, C], F32, kind="ExternalInput").ap()
        wqkv = nc.dram_tensor("wqkv", [C, 3 * CPC], F32, kind="ExternalInput").ap()
        bqkv = nc.dram_tensor("bqkv", [3 * CPC], F32, kind="ExternalInput").ap()
        wout = nc.dram_tensor("wout", [CPC, C], F32, kind="ExternalInput").ap()
        ones_d = nc.dram_tensor("ones_d", [128, 128], F32, kind="ExternalInput").ap()
        tri_d = nc.dram_tensor("tri_d", [128, 128], F32, kind="ExternalInput").ap()
        out = nc.dram_tensor("out", [T, C], F32, kind="ExternalOutput").ap()
        with tile.TileContext(nc) as tc:
            with ExitStack() as ctx:
                _emit(ctx, tc, xs, wqkv, bqkv, wout, ones_d, tri_d, out)
        nc.compile()
        nc.m = get_hw_module(nc.m)
        _COMPILED["nc"] = nc
    return _COMPILED["nc"]


def _in_maps(x, W_qkv, b_qkv, W_out):
    x = np.asarray(x, dtype=np.float32)
    W_qkv = np.asarray(W_qkv, dtype=np.float32)
    b_qkv = np.asarray(b_qkv, dtype=np.float32)
    W_out = np.asarray(W_out, dtype=np.float32)
    ones_np = np.ones((128, 128), dtype=np.float32)
    tri_np = np.triu(np.ones((128, 128), dtype=np.float32))
    maps = []
    for c in range(N_CORES):
        b, g = divmod(c, 2)
        sl = slice(g * CPC, (g + 1) * CPC)
        W_c = np.ascontiguousarray(
            np.concatenate(
                [W_qkv[:, sl], W_qkv[:, C:][:, sl], W_qkv[:, 2 * C :][:, sl]], axis=1
            )
        )
        b_c = np.ascontiguousarray(
            np.concatenate([b_qkv[sl], b_qkv[C:][sl], b_qkv[2 * C :][sl]])
        )
        maps.append(
            {
                "xs": np.ascontiguousarray(x[b]),
                "wqkv": W_c,
                "bqkv": b_c,
                "wout": np.ascontiguousarray(W_out[sl, :]),
                "ones_d": ones_np,
                "tri_d": tri_np,
            }
        )
    return maps


def _run(x, W_qkv, b_qkv, W_out, b_out, trace=False):
    nc = _get_compiled()
    res = run_bass_kernel_spmd(
        nc, _in_maps(x, W_qkv, b_qkv, W_out), list(range(N_CORES)), trace=trace
    )
    parts = [res.results[i]["out"] for i in range(N_CORES)]
    b_out = np.asarray(b_out, dtype=np.float32)
    full = np.stack([parts[2 * b] + parts[2 * b + 1] for b in range(B)]) + b_out
    return full.astype(np.float32), res


def kernel(x, W_qkv, b_qkv, W_out, b_out):
    full, _ = _run(x, W_qkv, b_qkv, W_out, b_out)
    return full


# revision 12
# speedup vs baseline: 1.1692x; 1.1692x over previous
"""Causal multi-head attention block (B=4, T=2048, C=1024, H=16, D=64) on 8 trn2 cores.

Sharding: core c handles batch b = c // 2 and heads g*8..g*8+8 where g = c % 2.
Each core computes the QKV projection restricted to its 8 heads, causal
attention, and a partial out-projection (contracting only its 512 head-dims).
The host sums the two partials per batch and adds the output bias.

On-device layout: attention is computed transposed -- S^T[k, q] = K @ Q^T --
so that the softmax sum over keys (the partition dim of S^T) folds into the
P@V matmul via an appended ones-column on V, and the attention output lands
as y^T [head_dim, T], which is exactly the lhsT layout the out-projection
matmul needs. Normalization uses a rank-1 outer-product broadcast of the
reciprocal denominators. Matmuls run in float32r (full PE rate at N>=256).
"""

import sys
from contextlib import ExitStack

import numpy as np

sys.path.insert(0, "/opt/trn_rl_repo")

import concourse.bass as bass  # noqa: F401
import concourse.mybir as mybir
import concourse.tile as tile
from concourse import bacc
from concourse.bass_interp import get_hw_module
from concourse.bass_utils import run_bass_kernel_spmd
from concourse.masks import make_identity

F32 = mybir.dt.float32
F32R = mybir.dt.float32r
AF = mybir.ActivationFunctionType

B, T, C, H, D = 4, 2048, 1024, 16, 64
HPC = 8            # heads per core
CPC = HPC * D      # 512 head-dim columns per core
N_CORES = 8
TB = T // 128      # 16 T row-blocks
KT = C // 128      # 8 contraction tiles over C
NQ = T // 512      # 4 q-chunks


def _emit(ctx, tc, xs, wqkv, bqkv, wout, ones_d, tri_d, out):
    nc = tc.nc

    const = ctx.enter_context(tc.tile_pool(name="const", bufs=1))
    identity = const.tile([128, 128], F32)
    make_identity(nc, identity)
    # tri[k, q] = 1.0 where k <= q else 0.0 (valid region of the causal mask in
    # transposed-score coordinates: keys on partitions, queries on free dim).
    # Loaded from the host: f32r-consumed constants must come from DMA.
    tri = const.tile([128, 128], F32R)
    nc.sync.dma_start(out=tri, in_=tri_d[:, :].bitcast(F32R))
    ones_s = const.tile([1, 128], F32R)
    nc.sync.dma_start(out=ones_s, in_=ones_d[0:1, :].bitcast(F32R))
    bias_qk = const.tile([128, 8], F32)
    nc.sync.dma_start(out=bias_qk, in_=bqkv[0 : 2 * CPC].rearrange("(m p) -> p m", p=128))
    bias_v = const.tile([1, CPC], F32R)
    nc.sync.dma_start(out=bias_v, in_=bqkv[2 * CPC :].rearrange("(a m) -> a m", a=1).bitcast(F32R))

    # Big reusable region: holds x^T during phases A/B, then y^T + W_out after.
    big_pool = ctx.enter_context(tc.tile_pool(name="big", bufs=1))
    # qk^T [1024 rows, T] as [128, 8, T]; rows 0..511 = q^T, 512..1023 = k^T
    qkT = ctx.enter_context(tc.tile_pool(name="qkT", bufs=1)).tile([128, 8, T], F32R)
    # v natural [T, 8 heads, 64+1] as [128, TB, 8, 65]; col 64 = ones
    v_all = ctx.enter_context(tc.tile_pool(name="vall", bufs=1)).tile(
        [128, TB, HPC, D + 1], F32R
    )

    xt = big_pool.tile([128, KT, T], F32R, tag="big", name="xt")

    # ---------- Phase A: load x and build x^T ----------
    with (
        tc.tile_pool(name="xnat", bufs=3) as xnat_pool,
        tc.tile_pool(name="ps_t", bufs=4, space="PSUM") as ps_t_pool,
    ):
        for tb in range(TB):
            xn = xnat_pool.tile([128, C], F32)
            nc.sync.dma_start(out=xn, in_=xs[tb * 128 : (tb + 1) * 128, :])
            for cb in range(KT):
                pst = ps_t_pool.tile([128, 128], F32)
                nc.tensor.transpose(pst, xn[:, cb * 128 : (cb + 1) * 128], identity)
                dest = xt[:, cb, tb * 128 : (tb + 1) * 128]
                if cb % 2 == 0:
                    nc.vector.tensor_copy(dest, pst)
                else:
                    nc.scalar.activation(dest, pst, AF.Copy)

    # ---------- Phase B: QKV projection ----------
    with (
        tc.tile_pool(name="wst", bufs=16) as w_pool,
        tc.tile_pool(name="wv", bufs=1) as wv_pool,
        tc.tile_pool(name="ps_mm", bufs=4, space="PSUM") as ps_mm_pool,
    ):
        # q^T / k^T: out rows = qk column index (per-partition), streamed over T.
        # kt-outer so 4 consecutive matmuls reuse each (f32r, non-FWL) weight load.
        for mb in range(8):
            w_tiles = []
            for kt in range(KT):
                w_t = w_pool.tile([128, 128], F32R, tag="w")
                nc.sync.dma_start(
                    out=w_t,
                    in_=wqkv[kt * 128 : (kt + 1) * 128, mb * 128 : (mb + 1) * 128].bitcast(F32R),
                )
                w_tiles.append(w_t)
            pss = [ps_mm_pool.tile([128, 512], F32, tag="qk", name=f"psqk{i}") for i in range(NQ)]
            for kt in range(KT):
                for nch in range(NQ):
                    nc.tensor.matmul(
                        pss[nch],
                        w_tiles[kt],
                        xt[:, kt, nch * 512 : (nch + 1) * 512],
                        start=(kt == 0),
                        stop=(kt == KT - 1),
                    )
            for nch in range(NQ):
                nc.vector.tensor_scalar_add(
                    qkT[:, mb, nch * 512 : (nch + 1) * 512],
                    pss[nch],
                    bias_qk[:, mb : mb + 1],
                )

        # v natural: rows = keys, cols = 8 heads x 64; bias added via K=1 matmul
        wv = wv_pool.tile([128, KT, CPC], F32R)
        for kt in range(KT):
            nc.sync.dma_start(
                out=wv[:, kt, :],
                in_=wqkv[kt * 128 : (kt + 1) * 128, 2 * CPC : 3 * CPC].bitcast(F32R),
            )
        nc.sync.dma_start(
            out=v_all[:, :, :, D : D + 1],
            in_=ones_d[:, 0 : TB * HPC]
            .rearrange("p (a b c) -> p a b c", a=TB, c=1)
            .bitcast(F32R),
        )
        for tb in range(TB):
            ps = ps_mm_pool.tile([128, 512], F32)
            for kt in range(KT):
                nc.tensor.matmul(
                    ps,
                    xt[:, kt, tb * 128 : (tb + 1) * 128],
                    wv[:, kt, :],
                    start=(kt == 0),
                    stop=False,
                )
            nc.tensor.matmul(
                ps, ones_s, bias_v, start=False, stop=True
            )
            for hh in range(HPC):
                dst = v_all[:, tb, hh, 0:D]
                srcc = ps[:, hh * D : (hh + 1) * D]
                if hh % 2 == 0:
                    nc.vector.tensor_copy(dst, srcc)
                else:
                    nc.scalar.activation(dst, srcc, AF.Copy)

    # x^T region is dead now; reuse it for y^T (rows = 512 head dims) + W_out.
    # combo[:, i, 0:T] = y^T tile i; combo[:, i, T:T+C] = W_out rows i*128..
    combo = big_pool.tile([128, 4, T + C], F32R, tag="big", name="combo")
    for kt in range(4):
        nc.sync.dma_start(
            out=combo[:, kt, T : T + C], in_=wout[kt * 128 : (kt + 1) * 128, :].bitcast(F32R)
        )

    # ---------- Phase C: attention (kb-major per head, grouped weight loads) ----------
    # f32r weight loads are serial with the matmul (no FWL), so consecutive
    # matmuls that share a stationary operand are grouped: per (head, key
    # block) one S-group streams all valid q-chunks off one k^T load, then one
    # PV-group streams them off one V load, accumulating into 4 parallel PSUM
    # banks (one per q-chunk).
    with (
        tc.tile_pool(name="e", bufs=8) as e_pool,
        tc.tile_pool(name="bc", bufs=2) as bc_pool,
        tc.tile_pool(name="rcp", bufs=4) as rcp_pool,
        tc.tile_pool(name="ost", bufs=4) as ost_pool,
        tc.tile_pool(name="ps_s", bufs=2, space="PSUM") as ps_s_pool,
        tc.tile_pool(name="ps_y", bufs=4, space="PSUM") as ps_y_pool,
        tc.tile_pool(name="ps_b", bufs=1, space="PSUM") as ps_b_pool,
    ):
        for h in range(HPC):
            po = (h % 2) * 64  # partition offset of this head's 64 rows
            qi = h // 2        # q^T tile index; k^T tile index is 4 + qi
            qT_row = lambda c0, c1: qkT[po : po + 64, qi, c0:c1]  # noqa: E731
            kT_row = lambda c0, c1: qkT[po : po + 64, 4 + qi, c0:c1]  # noqa: E731

            pys = [ps_y_pool.tile([65, 512], F32, tag="py", name=f"py{i}") for i in range(NQ)]
            for kb in range(4 * NQ):
                qc_min = kb // 4
                e_sbs = {}
                for qc in range(qc_min, NQ):
                    c0 = max(kb - qc * 4, 0) * 128
                    ps_s = ps_s_pool.tile([128, 512], F32, tag="s")
                    e_sb = e_pool.tile([128, 512], F32R, tag="e")
                    nc.tensor.matmul(
                        ps_s[:, c0:],
                        kT_row(kb * 128, kb * 128 + 128),
                        qT_row(qc * 512 + c0, (qc + 1) * 512),
                        start=True,
                        stop=True,
                    )
                    nc.scalar.activation(
                        e_sb[:, c0:], ps_s[:, c0:], AF.Exp, scale=0.125
                    )
                    if kb == qc * 4 + (c0 // 128):  # diagonal sub-block
                        nc.vector.tensor_mul(
                            e_sb[:, c0 : c0 + 128], e_sb[:, c0 : c0 + 128], tri
                        )
                    e_sbs[qc] = (e_sb, c0)
                for qc in range(qc_min, NQ):
                    e_sb, c0 = e_sbs[qc]
                    nc.tensor.matmul(
                        pys[qc][:, c0:],
                        v_all[:, kb, h, :],
                        e_sb[:, c0:],
                        start=(kb == 0),
                        stop=(kb == qc * 4 + 3),
                    )
                    if kb == qc * 4 + 3:
                        # q-chunk finished: y = py[0:64] / py[64]. Broadcast the
                        # raw denominator via a rank-1 matmul, then take the
                        # reciprocal on all 64 lanes at once (single DVE op).
                        zrow = rcp_pool.tile([1, 512], F32R, tag="z")
                        nc.vector.tensor_copy(zrow, pys[qc][64:65, :])
                        pb = ps_b_pool.tile([64, 512], F32, tag="b")
                        nc.tensor.matmul(
                            pb, ones_s[0:1, 0:64], zrow, start=True, stop=True
                        )
                        bcr = bc_pool.tile([64, 512], F32)
                        nc.vector.reciprocal_approx_fast(bcr, pb)
                        nc.vector.tensor_mul(
                            combo[po : po + 64, qi, qc * 512 : (qc + 1) * 512],
                            pys[qc][0:64, :],
                            bcr,
                        )

        # ---------- out-projection (tail; kt-outer so each y^T load serves both halves) ----------
        for tb in range(TB):
            pos = [ps_s_pool.tile([128, 512], F32, tag="s", name=f"po{i}") for i in range(2)]
            for kt in range(4):
                for nn in range(2):
                    nc.tensor.matmul(
                        pos[nn],
                        combo[:, kt, tb * 128 : (tb + 1) * 128],
                        combo[:, kt, T + nn * 512 : T + (nn + 1) * 512],
                        start=(kt == 0),
                        stop=(kt == 3),
                    )
            for nn in range(2):
                o_sb = ost_pool.tile([128, 512], F32, tag="o")
                nc.vector.tensor_copy(o_sb, pos[nn])
                nc.sync.dma_start(
                    out=out[tb * 128 : (tb + 1) * 128, nn * 512 : (nn + 1) * 512],
                    in_=o_sb,
                )


_COMPILED = {}


def _get_compiled():
    if "nc" not in _COMPILED:
        nc = bacc.Bacc(
            "TRN2",
            target_bir_lowering=False,
            debug=False,
            num_devices=N_CORES,
        )
        xs = nc.dram_tensor("xs", [T, C], F32, kind="ExternalInput").ap()
        wqkv = nc.dram_tensor("wqkv", [C, 3 * CPC], F32, kind="ExternalInput").ap()
        bqkv = nc.dram_tensor("bqkv", [3 * CPC], F32, kind="ExternalInput").ap()
        wout = nc.dram_tensor("wout", [CPC, C], F32, kind="ExternalInput").ap()
        ones_d = nc.dram_tensor("ones_d", [128, 128], F32, kind="ExternalInput").ap()
        tri_d = nc.dram_tensor("tri_d", [128, 128], F32, kind="ExternalInput").ap()
        out = nc.dram_tensor("out", [T, C], F32, kind="ExternalOutput").ap()
        with tile.TileContext(nc) as tc:
            with ExitStack() as ctx:
                _emit(ctx, tc, xs, wqkv, bqkv, wout, ones_d, tri_d, out)
        nc.compile()
        nc.m = get_hw_module(nc.m)
        _COMPILED["nc"] = nc
    return _COMPILED["nc"]


def _in_maps(x, W_qkv, b_qkv, W_out):
    x = np.asarray(x, dtype=np.float32)
    W_qkv = np.asarray(W_qkv, dtype=np.float32)
    b_qkv = np.asarray(b_qkv, dtype=np.float32)
    W_out = np.asarray(W_out, dtype=np.float32)
    ones_np = np.ones((128, 128), dtype=np.float32)
    tri_np = np.triu(np.ones((128, 128), dtype=np.float32))
    maps = []
    for c in range(N_CORES):
        b, g = divmod(c, 2)
        sl = slice(g * CPC, (g + 1) * CPC)
        W_c = np.ascontiguousarray(
            np.concatenate(
                [W_qkv[:, sl], W_qkv[:, C:][:, sl], W_qkv[:, 2 * C :][:, sl]], axis=1
            )
        )
        b_c = np.ascontiguousarray(
            np.concatenate([b_qkv[sl], b_qkv[C:][sl], b_qkv[2 * C :][sl]])
        )
        maps.append(
            {
                "xs": np.ascontiguousarray(x[b]),
                "wqkv": W_c,
                "bqkv": b_c,
                "wout": np.ascontiguousarray(W_out[sl, :]),
                "ones_d": ones_np,
                "tri_d": tri_np,
            }
        )
    return maps


def _run(x, W_qkv, b_qkv, W_out, b_out, trace=False):
    nc = _get_compiled()
    res = run_bass_kernel_spmd(
        nc, _in_maps(x, W_qkv, b_qkv, W_out), list(range(N_CORES)), trace=trace
    )
    parts = [res.results[i]["out"] for i in range(N_CORES)]
    b_out = np.asarray(b_out, dtype=np.float32)
    full = np.stack([parts[2 * b] + parts[2 * b + 1] for b in range(B)]) + b_out
    return full.astype(np.float32), res


def kernel(x, W_qkv, b_qkv, W_out, b_out):
    full, _ = _run(x, W_qkv, b_qkv, W_out, b_out)
    return full


# revision 21
# speedup vs baseline: 1.2245x; 1.0473x over previous
"""Causal multi-head attention block (B=4, T=2048, C=1024, H=16, D=64) on 8 trn2 cores.

Sharding: core c handles batch b = c // 2 and heads g*8..g*8+8 where g = c % 2.
Each core computes the QKV projection restricted to its 8 heads, causal
attention, and a partial out-projection (contracting only its 512 head-dims).
The host sums the two partials per batch and adds the output bias.

On-device layout: attention is computed transposed -- S^T[k, q] = K @ Q^T --
so that the softmax sum over keys (the partition dim of S^T) folds into the
P@V matmul via an appended ones-column on V, and the attention output lands
as y^T [head_dim, T], which is exactly the lhsT layout the out-projection
matmul needs. Normalization uses a rank-1 outer-product broadcast of the
reciprocal denominators. Matmuls run in float32r (full PE rate at N>=256).
"""

import sys
from contextlib import ExitStack

import numpy as np

sys.path.insert(0, "/opt/trn_rl_repo")

import concourse.bass as bass  # noqa: F401
import concourse.mybir as mybir
import concourse.tile as tile
from concourse import bacc
from concourse.bass_interp import get_hw_module
from concourse.bass_utils import run_bass_kernel_spmd
from concourse.masks import make_identity

F32 = mybir.dt.float32
F32R = mybir.dt.float32r
BF16 = mybir.dt.bfloat16
AF = mybir.ActivationFunctionType

B, T, C, H, D = 4, 2048, 1024, 16, 64
HPC = 8            # heads per core
CPC = HPC * D      # 512 head-dim columns per core
N_CORES = 8
TB = T // 128      # 16 T row-blocks
KT = C // 128      # 8 contraction tiles over C
NQ = T // 512      # 4 q-chunks


def _emit(ctx, tc, xs, wqkv, bqkv, wout, ones_d, trib, out):
    nc = tc.nc

    const = ctx.enter_context(tc.tile_pool(name="const", bufs=1))
    identity = const.tile([128, 128], F32)
    make_identity(nc, identity)
    # tri[k, q] = 1.0 where k <= q else 0.0 (valid region of the causal mask in
    # transposed-score coordinates: keys on partitions, queries on free dim).
    # Loaded from the host: f32r-consumed constants must come from DMA.
    tri = const.tile([128, 128], BF16)
    nc.sync.dma_start(out=tri, in_=trib[:, 0:128])
    ones_s = const.tile([1, 128], F32R)
    nc.sync.dma_start(out=ones_s, in_=ones_d[0:1, :].bitcast(F32R))
    bias_qk = const.tile([128, 8], F32)
    nc.sync.dma_start(out=bias_qk, in_=bqkv[0 : 2 * CPC].rearrange("(m p) -> p m", p=128))
    bias_v = const.tile([1, CPC], F32R)
    nc.sync.dma_start(out=bias_v, in_=bqkv[2 * CPC :].rearrange("(a m) -> a m", a=1).bitcast(F32R))

    # Big reusable region: holds x^T during phases A/B, then y^T + W_out after.
    big_pool = ctx.enter_context(tc.tile_pool(name="big", bufs=1))
    # qk^T [1024 rows, T] as [128, 8, T]; rows 0..511 = q^T, 512..1023 = k^T
    qkT = ctx.enter_context(tc.tile_pool(name="qkT", bufs=1)).tile([128, 8, T], BF16)
    # v natural [T, 8 heads, 64+1] as [128, TB, 8, 65]; col 64 = ones
    v_all = ctx.enter_context(tc.tile_pool(name="vall", bufs=1)).tile(
        [128, TB, HPC, D + 1], BF16
    )

    xt = big_pool.tile([128, KT, T], F32R, tag="big", name="xt")
    wv = ctx.enter_context(tc.tile_pool(name="wv", bufs=1)).tile(
        [128, KT, CPC], F32R
    )
    for kt in range(KT):
        nc.sync.dma_start(
            out=wv[:, kt, :],
            in_=wqkv[kt * 128 : (kt + 1) * 128, 2 * CPC : 3 * CPC].bitcast(F32R),
        )
    nc.sync.dma_start(
        out=v_all[:, :, :, D : D + 1],
        in_=trib[:, 128 : 128 + TB * HPC].rearrange(
            "p (a b c) -> p a b c", a=TB, c=1
        ),
    )

    # ---------- Phase A: load x and build x^T ----------
    with (
        tc.tile_pool(name="xnat", bufs=3) as xnat_pool,
        tc.tile_pool(name="ps_t", bufs=4, space="PSUM") as ps_t_pool,
    ):
        for tb in range(TB):
            xn = xnat_pool.tile([128, C], F32)
            nc.sync.dma_start(out=xn, in_=xs[tb * 128 : (tb + 1) * 128, :])
            for cb in range(KT):
                pst = ps_t_pool.tile([128, 128], F32)
                nc.tensor.transpose(pst, xn[:, cb * 128 : (cb + 1) * 128], identity)
                dest = xt[:, cb, tb * 128 : (tb + 1) * 128]
                if cb % 2 == 0:
                    nc.vector.tensor_copy(dest, pst)
                else:
                    nc.scalar.activation(dest, pst, AF.Copy)

    # ---------- Phase B: QKV projection ----------
    with (
        tc.tile_pool(name="wst", bufs=16) as w_pool,
        tc.tile_pool(name="ps_mm", bufs=4, space="PSUM") as ps_mm_pool,
    ):
        # q^T / k^T: out rows = qk column index (per-partition), streamed over T.
        # kt-outer so 4 consecutive matmuls reuse each (f32r, non-FWL) weight load.
        for mb in range(8):
            w_tiles = []
            for kt in range(KT):
                w_t = w_pool.tile([128, 128], F32R, tag="w")
                nc.sync.dma_start(
                    out=w_t,
                    in_=wqkv[kt * 128 : (kt + 1) * 128, mb * 128 : (mb + 1) * 128].bitcast(F32R),
                )
                w_tiles.append(w_t)
            pss = [ps_mm_pool.tile([128, 512], F32, tag="qk", name=f"psqk{i}") for i in range(NQ)]
            for kt in range(KT):
                for nch in range(NQ):
                    nc.tensor.matmul(
                        pss[nch],
                        w_tiles[kt],
                        xt[:, kt, nch * 512 : (nch + 1) * 512],
                        start=(kt == 0),
                        stop=(kt == KT - 1),
                    )
            for nch in range(NQ):
                nc.vector.tensor_scalar_add(
                    qkT[:, mb, nch * 512 : (nch + 1) * 512],
                    pss[nch],
                    bias_qk[:, mb : mb + 1],
                )


    # x^T region is dead now; reuse it for y^T (rows = 512 head dims) + W_out.
    # combo[:, i, 0:T] = y^T tile i; combo[:, i, T:T+C] = W_out rows i*128..
    combo = big_pool.tile([128, 4, T + C], F32R, tag="combo", name="combo")
    for kt in range(4):
        nc.sync.dma_start(
            out=combo[:, kt, T : T + C], in_=wout[kt * 128 : (kt + 1) * 128, :].bitcast(F32R)
        )

    # ---------- Phase C: attention, qc-major ----------
    # For each q-chunk: JIT-project V for the 4 new key blocks (PE filler),
    # run all 8 heads (independent S->exp->PV chains pipeline across heads
    # through a deep PSUM pool, keeping the PE dense / HAM-warm), then the
    # out-projection for the finished T rows.
    with (
        tc.tile_pool(name="e", bufs=8) as e_pool,
        tc.tile_pool(name="bc", bufs=2) as bc_pool,
        tc.tile_pool(name="rcp", bufs=4) as rcp_pool,
        tc.tile_pool(name="ost", bufs=4) as ost_pool,
        tc.tile_pool(name="ps_s", bufs=4, space="PSUM") as ps_s_pool,
        tc.tile_pool(name="ps_y", bufs=4, space="PSUM") as ps_y_pool,
    ):
        for qc in range(NQ):
            for kb in range(qc * 4, qc * 4 + 4):
                psv = ps_s_pool.tile([128, 512], F32, tag="s", name="psv")
                for kt in range(KT):
                    nc.tensor.matmul(
                        psv,
                        xt[:, kt, kb * 128 : (kb + 1) * 128],
                        wv[:, kt, :],
                        start=(kt == 0),
                        stop=False,
                    )
                nc.tensor.matmul(psv, ones_s, bias_v, start=False, stop=True)
                for hh in range(HPC):
                    dst = v_all[:, kb, hh, 0:D]
                    srcc = psv[:, hh * D : (hh + 1) * D]
                    if hh % 2 == 0:
                        nc.vector.tensor_copy(dst, srcc)
                    else:
                        nc.scalar.activation(dst, srcc, AF.Copy)

            for hp in range(HPC // 2):
                h0, h1 = 2 * hp, 2 * hp + 1
                qi = hp  # both heads of the pair live in q^T/k^T tile hp
                pys = {
                    h0: ps_y_pool.tile([65, 512], F32, tag="py", name="py0"),
                    h1: ps_y_pool.tile([65, 512], F32, tag="py", name="py1"),
                }
                # diagonal blocks first: their extra DVE mask hop overlaps the
                # long full-block S stream instead of stalling the PE tail
                kb_order = list(range(qc * 4, qc * 4 + 4)) + list(range(qc * 4))
                last = len(kb_order) - 1
                for i, kb in enumerate(kb_order):
                    c0 = max(kb - qc * 4, 0) * 128
                    e_sbs = {}
                    for h in (h0, h1):
                        po = (h % 2) * 64
                        ps_s = ps_s_pool.tile(
                            [128, 512], F32, tag="s", name="ps_s"
                        )
                        e_sb = e_pool.tile([128, 512], BF16, tag="e", name="e_sb")
                        nc.tensor.matmul(
                            ps_s[:, c0:],
                            qkT[po : po + 64, 4 + qi, kb * 128 : (kb + 1) * 128],
                            qkT[po : po + 64, qi, qc * 512 + c0 : (qc + 1) * 512],
                            start=True,
                            stop=True,
                        )
                        nc.scalar.activation(
                            e_sb[:, c0:], ps_s[:, c0:], AF.Exp, scale=0.125
                        )
                        if kb >= qc * 4:  # diagonal sub-block
                            nc.vector.tensor_mul(
                                e_sb[:, c0 : c0 + 128], e_sb[:, c0 : c0 + 128], tri
                            )
                        e_sbs[h] = (e_sb, c0)
                    for h in (h0, h1):
                        e_sb, c0 = e_sbs[h]
                        nc.tensor.matmul(
                            pys[h][:, c0:],
                            v_all[:, kb, h, :],
                            e_sb[:, c0:],
                            start=(i == 0),
                            stop=(i == last),
                        )

                for h in (h0, h1):
                    po = (h % 2) * 64
                    py = pys[h]
                    # y = py[0:64] / py[64]: broadcast the raw denominator with
                    # a rank-1 matmul, reciprocal on all 64 lanes in one DVE op.
                    # zrow goes via ACT so the PE's outer-product does not wait
                    # on the DVE queue.
                    zrow = rcp_pool.tile([1, 512], F32R, tag="z")
                    nc.scalar.activation(zrow, py[64:65, :], AF.Copy)
                    pb = ps_s_pool.tile([64, 512], F32, tag="s", name="pb")
                    nc.tensor.matmul(
                        pb, ones_s[0:1, 0:64], zrow, start=True, stop=True
                    )
                    bcr = bc_pool.tile([64, 512], F32)
                    nc.vector.reciprocal_approx_fast(bcr, pb)
                    nc.vector.tensor_mul(
                        combo[po : po + 64, qi, qc * 512 : (qc + 1) * 512],
                        py[0:64, :],
                        bcr,
                    )

            # out-projection for the 4 T-blocks this q-chunk completed
            for tb in range(qc * 4, qc * 4 + 4):
                pos = [
                    ps_s_pool.tile([128, 512], F32, tag="s", name=f"po{i}")
                    for i in range(2)
                ]
                for kt in range(4):
                    for nn in range(2):
                        nc.tensor.matmul(
                            pos[nn],
                            combo[:, kt, tb * 128 : (tb + 1) * 128],
                            combo[:, kt, T + nn * 512 : T + (nn + 1) * 512],
                            start=(kt == 0),
                            stop=(kt == 3),
                        )
                for nn in range(2):
                    o_sb = ost_pool.tile([128, 512], F32, tag="o")
                    nc.vector.tensor_copy(o_sb, pos[nn])
                    nc.sync.dma_start(
                        out=out[tb * 128 : (tb + 1) * 128, nn * 512 : (nn + 1) * 512],
                        in_=o_sb,
                    )


_COMPILED = {}


def _get_compiled():
    if "nc" not in _COMPILED:
        nc = bacc.Bacc(
            "TRN2",
            target_bir_lowering=False,
            debug=False,
            num_devices=N_CORES,
        )
        xs = nc.dram_tensor("xs", [T, C], F32, kind="ExternalInput").ap()
        wqkv = nc.dram_tensor("wqkv", [C, 3 * CPC], F32, kind="ExternalInput").ap()
        bqkv = nc.dram_tensor("bqkv", [3 * CPC], F32, kind="ExternalInput").ap()
        wout = nc.dram_tensor("wout", [CPC, C], F32, kind="ExternalInput").ap()
        ones_d = nc.dram_tensor("ones_d", [128, 128], F32, kind="ExternalInput").ap()
        trib = nc.dram_tensor("trib", [128, 256], mybir.dt.bfloat16, kind="ExternalInput").ap()
        out = nc.dram_tensor("out", [T, C], F32, kind="ExternalOutput").ap()
        with tile.TileContext(nc) as tc:
            with ExitStack() as ctx:
                _emit(ctx, tc, xs, wqkv, bqkv, wout, ones_d, trib, out)
        nc.compile()
        nc.m = get_hw_module(nc.m)
        _COMPILED["nc"] = nc
    return _COMPILED["nc"]


def _in_maps(x, W_qkv, b_qkv, W_out):
    x = np.asarray(x, dtype=np.float32)
    W_qkv = np.asarray(W_qkv, dtype=np.float32)
    b_qkv = np.asarray(b_qkv, dtype=np.float32)
    W_out = np.asarray(W_out, dtype=np.float32)
    import ml_dtypes

    ones_np = np.ones((128, 128), dtype=np.float32)
    trib_np = np.ones((128, 256), dtype=np.float32)
    trib_np[:, 0:128] = np.triu(trib_np[:, 0:128])
    trib_np = trib_np.astype(ml_dtypes.bfloat16)
    maps = []
    for c in range(N_CORES):
        b, g = divmod(c, 2)
        sl = slice(g * CPC, (g + 1) * CPC)
        W_c = np.ascontiguousarray(
            np.concatenate(
                [W_qkv[:, sl], W_qkv[:, C:][:, sl], W_qkv[:, 2 * C :][:, sl]], axis=1
            )
        )
        b_c = np.ascontiguousarray(
            np.concatenate([b_qkv[sl], b_qkv[C:][sl], b_qkv[2 * C :][sl]])
        )
        maps.append(
            {
                "xs": np.ascontiguousarray(x[b]),
                "wqkv": W_c,
                "bqkv": b_c,
                "wout": np.ascontiguousarray(W_out[sl, :]),
                "ones_d": ones_np,
                "trib": trib_np,
            }
        )
    return maps


def _run(x, W_qkv, b_qkv, W_out, b_out, trace=False):
    nc = _get_compiled()
    res = run_bass_kernel_spmd(
        nc, _in_maps(x, W_qkv, b_qkv, W_out), list(range(N_CORES)), trace=trace
    )
    parts = [res.results[i]["out"] for i in range(N_CORES)]
    b_out = np.asarray(b_out, dtype=np.float32)
    full = np.stack([parts[2 * b] + parts[2 * b + 1] for b in range(B)]) + b_out
    return full.astype(np.float32), res


def kernel(x, W_qkv, b_qkv, W_out, b_out):
    full, _ = _run(x, W_qkv, b_qkv, W_out, b_out)
    return full


# revision 22
# speedup vs baseline: 1.2764x; 1.0424x over previous
"""Causal multi-head attention block (B=4, T=2048, C=1024, H=16, D=64) on 8 trn2 cores.

Sharding: core c handles batch b = c // 2 and heads g*8..g*8+8 where g = c % 2.
Each core computes the QKV projection restricted to its 8 heads, causal
attention, and a partial out-projection (contracting only its 512 head-dims).
The host sums the two partials per batch and adds the output bias.

On-device layout: attention is computed transposed -- S^T[k, q] = K @ Q^T --
so that the softmax sum over keys (the partition dim of S^T) folds into the
P@V matmul via an appended ones-column on V, and the attention output lands
as y^T [head_dim, T], which is exactly the lhsT layout the out-projection
matmul needs. Normalization uses a rank-1 outer-product broadcast of the
reciprocal denominators. Matmuls run in float32r (full PE rate at N>=256).
"""

import sys
from contextlib import ExitStack

import numpy as np

sys.path.insert(0, "/opt/trn_rl_repo")

import concourse.bass as bass  # noqa: F401
import concourse.mybir as mybir
import concourse.tile as tile
from concourse import bacc
from concourse.bass_interp import get_hw_module
from concourse.bass_utils import run_bass_kernel_spmd
from concourse.masks import make_identity

F32 = mybir.dt.float32
F32R = mybir.dt.float32r
BF16 = mybir.dt.bfloat16
AF = mybir.ActivationFunctionType

B, T, C, H, D = 4, 2048, 1024, 16, 64
HPC = 8            # heads per core
CPC = HPC * D      # 512 head-dim columns per core
N_CORES = 8
TB = T // 128      # 16 T row-blocks
KT = C // 128      # 8 contraction tiles over C
NQ = T // 512      # 4 q-chunks


def _emit(ctx, tc, xs, wqkv, bqkv, wout, ones_d, trib, out):
    nc = tc.nc

    const = ctx.enter_context(tc.tile_pool(name="const", bufs=1))
    identity = const.tile([128, 128], F32)
    make_identity(nc, identity)
    # tri[k, q] = 1.0 where k <= q else 0.0 (valid region of the causal mask in
    # transposed-score coordinates: keys on partitions, queries on free dim).
    # Loaded from the host: f32r-consumed constants must come from DMA.
    tri = const.tile([128, 128], BF16)
    nc.sync.dma_start(out=tri, in_=trib[:, 0:128])
    ones_s = const.tile([1, 128], F32R)
    nc.sync.dma_start(out=ones_s, in_=ones_d[0:1, :].bitcast(F32R))
    bias_qk = const.tile([128, 8], F32)
    nc.sync.dma_start(out=bias_qk, in_=bqkv[0 : 2 * CPC].rearrange("(m p) -> p m", p=128))
    bias_v = const.tile([1, CPC], F32R)
    nc.sync.dma_start(out=bias_v, in_=bqkv[2 * CPC :].rearrange("(a m) -> a m", a=1).bitcast(F32R))

    # Big reusable region: holds x^T during phases A/B, then y^T + W_out after.
    big_pool = ctx.enter_context(tc.tile_pool(name="big", bufs=1))
    # qk^T [1024 rows, T] as [128, 8, T]; rows 0..511 = q^T, 512..1023 = k^T
    qkT = ctx.enter_context(tc.tile_pool(name="qkT", bufs=1)).tile([128, 8, T], BF16)
    # v natural [T, 8 heads, 64+1] as [128, TB, 8, 65]; col 64 = ones
    v_all = ctx.enter_context(tc.tile_pool(name="vall", bufs=1)).tile(
        [128, TB, HPC, D + 1], BF16
    )

    xt = big_pool.tile([128, KT, T], F32R, tag="big", name="xt")
    wv = ctx.enter_context(tc.tile_pool(name="wv", bufs=1)).tile(
        [128, KT, CPC], F32R
    )
    for kt in range(KT):
        nc.sync.dma_start(
            out=wv[:, kt, :],
            in_=wqkv[kt * 128 : (kt + 1) * 128, 2 * CPC : 3 * CPC].bitcast(F32R),
        )
    nc.sync.dma_start(
        out=v_all[:, :, :, D : D + 1],
        in_=trib[:, 128 : 128 + TB * HPC].rearrange(
            "p (a b c) -> p a b c", a=TB, c=1
        ),
    )

    # ---------- Phase A: load x and build x^T ----------
    with (
        tc.tile_pool(name="xnat", bufs=3) as xnat_pool,
        tc.tile_pool(name="ps_t", bufs=4, space="PSUM") as ps_t_pool,
    ):
        for tb in range(TB):
            xn = xnat_pool.tile([128, C], F32)
            nc.sync.dma_start(out=xn, in_=xs[tb * 128 : (tb + 1) * 128, :])
            for cb in range(KT):
                pst = ps_t_pool.tile([128, 128], F32)
                nc.tensor.transpose(pst, xn[:, cb * 128 : (cb + 1) * 128], identity)
                dest = xt[:, cb, tb * 128 : (tb + 1) * 128]
                if cb % 2 == 0:
                    nc.vector.tensor_copy(dest, pst)
                else:
                    nc.scalar.activation(dest, pst, AF.Copy)

    # ---------- Phase B: QKV projection ----------
    with (
        tc.tile_pool(name="wst", bufs=16) as w_pool,
        tc.tile_pool(name="ps_mm", bufs=4, space="PSUM") as ps_mm_pool,
    ):
        # q^T / k^T: out rows = qk column index (per-partition), streamed over T.
        # kt-outer so 4 consecutive matmuls reuse each (f32r, non-FWL) weight load.
        for mb in range(8):
            w_tiles = []
            for kt in range(KT):
                w_t = w_pool.tile([128, 128], F32R, tag="w")
                nc.sync.dma_start(
                    out=w_t,
                    in_=wqkv[kt * 128 : (kt + 1) * 128, mb * 128 : (mb + 1) * 128].bitcast(F32R),
                )
                w_tiles.append(w_t)
            pss = [ps_mm_pool.tile([128, 512], F32, tag="qk", name=f"psqk{i}") for i in range(NQ)]
            for kt in range(KT):
                for nch in range(NQ):
                    nc.tensor.matmul(
                        pss[nch],
                        w_tiles[kt],
                        xt[:, kt, nch * 512 : (nch + 1) * 512],
                        start=(kt == 0),
                        stop=(kt == KT - 1),
                    )
            for nch in range(NQ):
                nc.vector.tensor_scalar_add(
                    qkT[:, mb, nch * 512 : (nch + 1) * 512],
                    pss[nch],
                    bias_qk[:, mb : mb + 1],
                )


    # x^T region is dead now; reuse it for y^T (rows = 512 head dims) + W_out.
    # combo[:, i, 0:T] = y^T tile i; combo[:, i, T:T+C] = W_out rows i*128..
    combo = big_pool.tile([128, 4, T + C], F32R, tag="combo", name="combo")
    for kt in range(4):
        nc.sync.dma_start(
            out=combo[:, kt, T : T + C], in_=wout[kt * 128 : (kt + 1) * 128, :].bitcast(F32R)
        )

    # ---------- Phase C: attention, qc-major ----------
    # For each q-chunk: JIT-project V for the 4 new key blocks (PE filler),
    # run all 8 heads (independent S->exp->PV chains pipeline across heads
    # through a deep PSUM pool, keeping the PE dense / HAM-warm), then the
    # out-projection for the finished T rows.
    with (
        tc.tile_pool(name="e", bufs=8) as e_pool,
        tc.tile_pool(name="bc", bufs=2) as bc_pool,
        tc.tile_pool(name="rcp", bufs=4) as rcp_pool,
        tc.tile_pool(name="ost", bufs=4) as ost_pool,
        tc.tile_pool(name="ps_s", bufs=6, space="PSUM") as ps_s_pool,
        tc.tile_pool(name="ps_y", bufs=2, space="PSUM") as ps_y_pool,
    ):
        for qc in range(NQ):
            for kb in range(qc * 4, qc * 4 + 4):
                psv = ps_s_pool.tile([128, 512], F32, tag="s", name="psv")
                for kt in range(KT):
                    nc.tensor.matmul(
                        psv,
                        xt[:, kt, kb * 128 : (kb + 1) * 128],
                        wv[:, kt, :],
                        start=(kt == 0),
                        stop=False,
                    )
                nc.tensor.matmul(psv, ones_s, bias_v, start=False, stop=True)
                for hh in range(HPC):
                    dst = v_all[:, kb, hh, 0:D]
                    srcc = psv[:, hh * D : (hh + 1) * D]
                    if hh % 2 == 0:
                        nc.vector.tensor_copy(dst, srcc)
                    else:
                        nc.scalar.activation(dst, srcc, AF.Copy)

            for h in range(HPC):
                po = (h % 2) * 64
                qi = h // 2
                qT_row = lambda c0, c1: qkT[po : po + 64, qi, c0:c1]  # noqa: E731
                kT_row = lambda c0, c1: qkT[po : po + 64, 4 + qi, c0:c1]  # noqa: E731

                py = ps_y_pool.tile([65, 512], F32, tag="py", name="py")
                n_kb = qc * 4 + 4

                def emit_s(kb):
                    c0 = max(kb - qc * 4, 0) * 128
                    ps_s = ps_s_pool.tile([128, 512], F32, tag="s", name="ps_s")
                    e_sb = e_pool.tile([128, 512], BF16, tag="e", name="e_sb")
                    nc.tensor.matmul(
                        ps_s[:, c0:],
                        kT_row(kb * 128, kb * 128 + 128),
                        qT_row(qc * 512 + c0, (qc + 1) * 512),
                        start=True,
                        stop=True,
                    )
                    nc.scalar.activation(
                        e_sb[:, c0:], ps_s[:, c0:], AF.Exp, scale=0.125
                    )
                    if kb >= qc * 4:  # diagonal sub-block
                        nc.vector.tensor_mul(
                            e_sb[:, c0 : c0 + 128], e_sb[:, c0 : c0 + 128], tri
                        )
                    return e_sb, c0

                def emit_pv(kb, e_sb, c0):
                    nc.tensor.matmul(
                        py[:, c0:],
                        v_all[:, kb, h, :],
                        e_sb[:, c0:],
                        start=(kb == 0),
                        stop=(kb == n_kb - 1),
                    )

                # software-pipeline: keep the S stream 2 key-blocks ahead of
                # the dependent PV so the in-order PE never waits on an exp
                pend = {}
                for kb in range(min(2, n_kb)):
                    pend[kb] = emit_s(kb)
                for kb in range(n_kb):
                    if kb + 2 < n_kb:
                        pend[kb + 2] = emit_s(kb + 2)
                    emit_pv(kb, *pend.pop(kb))

                # y = py[0:64] / py[64]: broadcast the raw denominator with a
                # rank-1 matmul, reciprocal on all 64 lanes in one DVE op
                zrow = rcp_pool.tile([1, 512], F32R, tag="z")
                nc.vector.tensor_copy(zrow, py[64:65, :])
                pb = ps_s_pool.tile([64, 512], F32, tag="s", name="pb")
                nc.tensor.matmul(pb, ones_s[0:1, 0:64], zrow, start=True, stop=True)
                bcr = bc_pool.tile([64, 512], F32)
                nc.vector.reciprocal_approx_fast(bcr, pb)
                nc.vector.tensor_mul(
                    combo[po : po + 64, qi, qc * 512 : (qc + 1) * 512],
                    py[0:64, :],
                    bcr,
                )

            # out-projection for the 4 T-blocks this q-chunk completed
            for tb in range(qc * 4, qc * 4 + 4):
                pos = [
                    ps_s_pool.tile([128, 512], F32, tag="s", name=f"po{i}")
                    for i in range(2)
                ]
                for kt in range(4):
                    for nn in range(2):
                        nc.tensor.matmul(
                            pos[nn],
                            combo[:, kt, tb * 128 : (tb + 1) * 128],
                            combo[:, kt, T + nn * 512 : T + (nn + 1) * 512],
                            start=(kt == 0),
                            stop=(kt == 3),
                        )
                for nn in range(2):
                    o_sb = ost_pool.tile([128, 512], F32, tag="o")
                    nc.vector.tensor_copy(o_sb, pos[nn])
                    nc.sync.dma_start(
                        out=out[tb * 128 : (tb + 1) * 128, nn * 512 : (nn + 1) * 512],
                        in_=o_sb,
                    )


_COMPILED = {}


def _get_compiled():
    if "nc" not in _COMPILED:
        nc = bacc.Bacc(
            "TRN2",
            target_bir_lowering=False,
            debug=False,
            num_devices=N_CORES,
        )
        xs = nc.dram_tensor("xs", [T, C], F32, kind="ExternalInput").ap()
        wqkv = nc.dram_tensor("wqkv", [C, 3 * CPC], F32, kind="ExternalInput").ap()
        bqkv = nc.dram_tensor("bqkv", [3 * CPC], F32, kind="ExternalInput").ap()
        wout = nc.dram_tensor("wout", [CPC, C], F32, kind="ExternalInput").ap()
        ones_d = nc.dram_tensor("ones_d", [128, 128], F32, kind="ExternalInput").ap()
        trib = nc.dram_tensor("trib", [128, 256], mybir.dt.bfloat16, kind="ExternalInput").ap()
        out = nc.dram_tensor("out", [T, C], F32, kind="ExternalOutput").ap()
        with tile.TileContext(nc) as tc:
            with ExitStack() as ctx:
                _emit(ctx, tc, xs, wqkv, bqkv, wout, ones_d, trib, out)
        nc.compile()
        nc.m = get_hw_module(nc.m)
        _COMPILED["nc"] = nc
    return _COMPILED["nc"]


def _in_maps(x, W_qkv, b_qkv, W_out):
    x = np.asarray(x, dtype=np.float32)
    W_qkv = np.asarray(W_qkv, dtype=np.float32)
    b_qkv = np.asarray(b_qkv, dtype=np.float32)
    W_out = np.asarray(W_out, dtype=np.float32)
    import ml_dtypes

    ones_np = np.ones((128, 128), dtype=np.float32)
    trib_np = np.ones((128, 256), dtype=np.float32)
    trib_np[:, 0:128] = np.triu(trib_np[:, 0:128])
    trib_np = trib_np.astype(ml_dtypes.bfloat16)
    maps = []
    for c in range(N_CORES):
        b, g = divmod(c, 2)
        sl = slice(g * CPC, (g + 1) * CPC)
        W_c = np.ascontiguousarray(
            np.concatenate(
                [W_qkv[:, sl], W_qkv[:, C:][:, sl], W_qkv[:, 2 * C :][:, sl]], axis=1
            )
        )
        b_c = np.ascontiguousarray(
            np.concatenate([b_qkv[sl], b_qkv[C:][sl], b_qkv[2 * C :][sl]])
        )
        maps.append(
            {
                "xs": np.ascontiguousarray(x[b]),
                "wqkv": W_c,
                "bqkv": b_c,
                "wout": np.ascontiguousarray(W_out[sl, :]),
                "ones_d": ones_np,
                "trib": trib_np,
            }
        )
    return maps


def _run(x, W_qkv, b_qkv, W_out, b_out, trace=False):
    nc = _get_compiled()
    res = run_bass_kernel_spmd(
        nc, _in_maps(x, W_qkv, b_qkv, W_out), list(range(N_CORES)), trace=trace
    )
    parts = [res.results[i]["out"] for i in range(N_CORES)]
    b_out = np.asarray(b_out, dtype=np.float32)
    full = np.stack([parts[2 * b] + parts[2 * b + 1] for b in range(B)]) + b_out
    return full.astype(np.float32), res


def kernel(x, W_qkv, b_qkv, W_out, b_out):
    full, _ = _run(x, W_qkv, b_qkv, W_out, b_out)
    return full
